# revision 1
# baseline (speedup 1.0000x reference)
"""CBAM3D Trainium2 kernel: 8-core SPMD, D-sharded, fp32 I/O with f32r matmuls.

x [2, 64, 64, 64, 64] f32. Each core owns an 8-plane D-slab, SBUF-resident as
[(b,c)=128 partitions, d*hw free], rounded to float32r on load so every PE
matmul streams at 1 cycle/row (vs 4 for fp32; measured rel err ~1.5e-4).

Phases per core:
 1. DMA planes -> staging; ScalarE Copy+accum_out writes the f32r slab and the
    per-(b,c) spatial sum; DVE reduce_max does the spatial max. 1 KB stats
    AllGather; MLP (block-diag PE matmuls) -> ca; ca folded into the cat-mean
    masks, a duplicated diagonal, and the gate selector (x stays unscaled).
 2. Channel-max: per 128-col chunk, PE matmul x_chunk.T @ [diag(ca)|diag(ca)]
    (N=256 keeps f32r fast) -> PSUM, DVE strided reduce_max over c. Channel-
    mean: PE block-diag matmuls accumulated in PSUM quarters, interleaved on
    the PE under the DVE-bound reduces. catmax lanes transposed back via PE.
 3. 524 KB cat AllGather (Shared); per-core one-hot window-select matmul
    (K=128 split in partition quarters) -> 14-plane padded window, duplicated
    w-shifted so conv packs 2 w-taps per matmul (K=120).
 4. 7x7x7 conv as 28 shifted-AP PE matmuls per 512-col chunk, chunk-major so
    each chunk flows straight into sigmoid (ScalarE), the sigma-broadcast
    gate matmul (bsel, ca-folded), the DVE multiply with x, and the store.
Self-contained: hardcodes shapes/sharding; inputs are repacked host-side.
"""
import numpy as np

import concourse.bass as bass
import concourse.mybir as mybir
import concourse.tile as tile
import concourse.bacc as bacc
import concourse.tile_utils as tile_utils

# allow a bit more SBUF than the stale default cap (224 KiB physical / 208 usable)
tile_utils.max_sbuf_usage = 204 * 1024

F32 = mybir.dt.float32
F32R = mybir.dt.float32r
ALU = mybir.AluOpType
ACTF = mybir.ActivationFunctionType

NCORES = 8
B, C, D, H, W = 2, 64, 64, 64, 64
HW = H * W                      # 4096
DL = D // NCORES                # 8 planes per core
R = C // 8                      # 8 reduced channels
KS, PAD = 7, 3
DE = DL + 2 * PAD               # 14 extended planes per core window
NSP = DL * HW                   # spatial elems per (b,c) per core = 32768
NTAPS = KS * KS                 # 49 (dh, dw) taps
NPAIR = KS * 4                  # 28 paired taps (dw pairs of 2, K=112)
NCHUNK = HW // 512              # 8 chunks of 512 per plane

_CACHED = {}


def _build_nc(timing=False, reps=1):
    ndev = 1 if timing else NCORES
    nc = bacc.Bacc("TRN2", target_bir_lowering=False, debug=False, num_devices=ndev)

    # ---- I/O ----
    xin = nc.dram_tensor("xin", [B, C, DL, H, W], F32, kind="ExternalInput")
    w1blk = nc.dram_tensor("w1blk", [128, 16], F32, kind="ExternalInput")
    w2blk = nc.dram_tensor("w2blk", [16, 128], F32, kind="ExternalInput")
    wconv = nc.dram_tensor("wconv", [120, NPAIR * 16], F32, kind="ExternalInput")
    seldr = nc.dram_tensor("sel", [128, 2 * 120], F32, kind="ExternalInput")
    cmaskdr = nc.dram_tensor("cmask", [128, 8 * 16], F32, kind="ExternalInput")
    gsumdr = nc.dram_tensor("gsum", [128, 16], F32, kind="ExternalInput")
    identdr = nc.dram_tensor("ident", [128, 256], F32, kind="ExternalInput")
    bseldr = nc.dram_tensor("bsel", [16, 8 * 128], F32, kind="ExternalInput")
    y = nc.dram_tensor("y", [B, C, DL, H, W], F32, kind="ExternalOutput")

    # DRAM scratch for collectives
    st_dram = nc.dram_tensor("st_dram", [128, 2], F32)
    st_gath = nc.dram_tensor("st_gath", [NCORES, 128, 2], F32, addr_space="Shared")
    cat_local = nc.dram_tensor("cat_local", [2, B, DL, HW], F32)  # (stat, b, d, hw)
    cat_gath = nc.dram_tensor("cat_gath", [NCORES, 2, B, DL, HW], F32, addr_space="Shared")

    xv = xin[:].rearrange("b c d h w -> (b c) d (h w)")   # [128, 8, 4096]
    yv = y[:].rearrange("b c d h w -> (b c) d (h w)")

    with tile.TileContext(nc, num_cores=NCORES) as tc:
        with (
            tc.tile_pool(name="persist", bufs=1) as pp,
            tc.tile_pool(name="dma_w", bufs=1) as pw,
        ):
            # ---- persistent SBUF ----
            x_sb = pp.tile([128, DL, HW], F32)            # 128 KiB/part
            w1_sb = pw.tile([128, 16], F32)
            w2_sb = pw.tile([16, 128], F32)
            nc.sync.dma_start(w1_sb[:], w1blk[:, :])
            nc.sync.dma_start(w2_sb[:], w2blk[:, :])
            # f32r-rounded PE operands (values are 0/1/w so rounding is exact)
            wc_r = pw.tile([120, NPAIR * 16], F32R)
            sel_r = pw.tile([128, 2 * 120], F32R)
            id_sb = pw.tile([128, 128], F32)              # for PE transpose
            # originals needed later for on-device ca folding
            cm_sb = pw.tile([128, 8 * 16], F32)
            id2_sb = pw.tile([128, 256], F32)
            bsel_sb = pw.tile([16, 8 * 128], F32)
            ones16 = pw.tile([1, 16], F32)
            nc.sync.dma_start(cm_sb[:], cmaskdr[:, :])
            nc.sync.dma_start(id2_sb[:], identdr[:, :])
            nc.sync.dma_start(bsel_sb[:], bseldr[:, :])
            nc.gpsimd.memset(ones16[:], 1.0)
            # ca-folded f32r operands, built after ca is known
            cmca_r = pw.tile([128, 8 * 16], F32R)
            dca2_r = pw.tile([128, 256], F32R)
            bselca_r = pw.tile([16, 8 * 128], F32R)
            car16 = pw.tile([16, 128], F32)
            with tc.tile_pool(name="worig", bufs=1) as pwo:
                wc_sb = pwo.tile([120, NPAIR * 16], F32)
                sel_sb = pwo.tile([128, 2 * 120], F32)
                nc.sync.dma_start(wc_sb[:], wconv[:, :])
                nc.sync.dma_start(sel_sb[:], seldr[:, :])
                nc.vector.tensor_copy(wc_r[:], wc_sb[:])
                nc.vector.tensor_copy(sel_r[:], sel_sb[:])
                nc.vector.tensor_copy(id_sb[:], id2_sb[:, 0:128])

            stat_sum = pp.tile([128, DL], F32)
            stat_max = pp.tile([128, DL], F32)
            ca_col = pp.tile([128, 1], F32)
            catmax = pp.tile([128, B, DL, 32], F32)       # (b, d, t) per s-lane
            sig_sb = pp.tile([16, HW], F32R)               # sigmoid(sa) rows (b, d)

            for _rep in range(reps):
                # ================= phase 1: load x + channel-attn stats =================
                # DMA lands in a staging tile; the ACT pass writes x_sb rounded to
                # f32r (so PE matmuls run at 1 cycle/row) and accumulates the
                # per-plane sum; DVE max reads the raw staging tile in parallel.
                with tc.tile_pool(name="p1io", bufs=2) as p1p:
                    for d in range(DL):
                        xl = p1p.tile([128, HW], F32, tag="xl")
                        # 4 sub-DMAs per plane -> more DMA-queue parallelism
                        for sq in range(4):
                            nc.sync.dma_start(
                                xl[:, sq * 1024 : (sq + 1) * 1024],
                                xv[:, d, sq * 1024 : (sq + 1) * 1024],
                            )
                        nc.scalar.activation(
                            x_sb[:, d, :].bitcast(F32R), xl[:],
                            ACTF.Copy, accum_out=stat_sum[:, d : d + 1],
                        )
                        nc.vector.tensor_reduce(
                            stat_max[:, d : d + 1], xl[:],
                            axis=mybir.AxisListType.X, op=ALU.max,
                        )
                    st2 = p1p.tile([128, 2], F32)
                    nc.vector.tensor_reduce(
                        st2[:, 0:1], stat_sum[:], axis=mybir.AxisListType.X, op=ALU.add
                    )
                    nc.vector.tensor_reduce(
                        st2[:, 1:2], stat_max[:], axis=mybir.AxisListType.X, op=ALU.max
                    )
                    nc.sync.dma_start(st_dram[:, :], st2[:])

                if not timing:
                    nc.gpsimd.collective_compute(
                        "AllGather", ALU.bypass,
                        replica_groups=[list(range(NCORES))],
                        ins=[st_dram[:].opt()], outs=[st_gath[:].opt()],
                    )

                # ================= phase 2: combine stats + MLP -> ca =================
                with (
                    tc.tile_pool(name="mlpsb", bufs=1) as mp,
                    tc.tile_pool(name="mlpps", bufs=1, space="PSUM") as mpp,
                ):
                    gst = mp.tile([128, NCORES, 2], F32)
                    nc.sync.dma_start(gst[:], st_gath[:].rearrange("n p s -> p n s"))
                    avg_col = mp.tile([128, 1], F32)
                    gmax_col = mp.tile([128, 1], F32)
                    nc.vector.tensor_reduce(
                        avg_col[:], gst[:].rearrange("p n s -> p s n")[:, 0:1, :],
                        axis=mybir.AxisListType.X, op=ALU.add,
                    )
                    nc.vector.tensor_reduce(
                        gmax_col[:], gst[:].rearrange("p n s -> p s n")[:, 1:2, :],
                        axis=mybir.AxisListType.X, op=ALU.max,
                    )
                    # scale sum -> mean
                    nc.scalar.mul(avg_col[:], avg_col[:], 1.0 / float(D * HW))

                    ps1 = mpp.tile([16, 2], F32)
                    nc.tensor.matmul(ps1[:, 0:1], w1_sb[:], avg_col[:], start=True, stop=True)
                    nc.tensor.matmul(ps1[:, 1:2], w1_sb[:], gmax_col[:], start=True, stop=True)
                    r_sb = mp.tile([16, 2], F32)
                    nc.scalar.activation(r_sb[:], ps1[:], ACTF.Relu)
                    ps2 = mpp.tile([128, 2], F32)
                    nc.tensor.matmul(ps2[:], w2_sb[:], r_sb[:], start=True, stop=True)
                    z2_sb = mp.tile([128, 2], F32)
                    nc.scalar.copy(z2_sb[:], ps2[:])
                    z_sb = mp.tile([128, 1], F32)
                    nc.vector.tensor_add(z_sb[:], z2_sb[:, 0:1], z2_sb[:, 1:2])
                    nc.scalar.activation(ca_col[:], z_sb[:], ACTF.Sigmoid)

                # fold ca into the PE-side operands (x itself stays unscaled f32r)
                with tc.tile_pool(name="caps", bufs=1, space="PSUM") as cps_pool:
                    nc.vector.tensor_scalar(
                        cmca_r[:], cm_sb[:], ca_col[:], None, op0=ALU.mult
                    )
                    nc.vector.tensor_scalar(
                        dca2_r[:], id2_sb[:], ca_col[:], None, op0=ALU.mult
                    )
                    # ca as a row, then broadcast to 16 partitions
                    pcar = cps_pool.tile([1, 128], F32, tag="pcar")
                    nc.tensor.matmul(
                        pcar[:], ca_col[:], id2_sb[:, 0:128], start=True, stop=True
                    )
                    car1 = pw.tile([1, 128], F32)
                    nc.scalar.copy(car1[:], pcar[:])
                    pcar16 = cps_pool.tile([16, 128], F32, tag="pcar16")
                    nc.tensor.matmul(pcar16[:], ones16[:], car1[:], start=True, stop=True)
                    nc.scalar.copy(car16[:], pcar16[:])
                    for db in range(DL):
                        nc.vector.tensor_tensor(
                            bselca_r[:, db * 128 : (db + 1) * 128],
                            bsel_sb[:, db * 128 : (db + 1) * 128],
                            car16[:],
                            op=ALU.mult,
                        )

                # ===== phase 3: xcT channel-max, assembly interleaved, then cat-mean =====
                # open the window pool early so its memset overlaps this phase
                _wsp_cm = tc.tile_pool(name="winsb", bufs=1)
                wsp = _wsp_cm.__enter__()
                win_sb = wsp.tile([120, H + 2 * PAD, W + 2 * PAD], F32R)  # 70x70 padded
                nc.gpsimd.memset(win_sb[:].bitcast(F32), 0.0)

                cmx_flat = catmax[:].rearrange("p b d t -> p (b d t)")  # [128, 512]
                with (
                    tc.tile_pool(name="xctps", bufs=2, space="PSUM") as xcp,
                    tc.tile_pool(name="trps", bufs=2, space="PSUM") as trp,
                    tc.tile_pool(name="cmq", bufs=1, space="PSUM") as cmq,
                    tc.tile_pool(name="trsb", bufs=2) as trs,
                ):
                    cmn_sb = wsp.tile([16, HW], F32, tag="chalf")
                    def assemble(j):
                        # catmax cols j*128.. are (b,d,t) rows; transpose to hw-major
                        ptr = trp.tile([128, 128], F32, tag="tr")
                        nc.tensor.transpose(
                            ptr[:], cmx_flat[:, j * 128 : (j + 1) * 128], id_sb[:, 0:128]
                        )
                        tsb = trs.tile([128, 128], F32, tag="trsb")
                        nc.scalar.copy(tsb[:], ptr[:])
                        nc.sync.dma_start(
                            cat_local[1:2, :, :, :].rearrange("o b d f -> (o b d) f")[
                                :, :
                            ].rearrange("r (t p) -> (r t) p", t=32)[
                                j * 128 : (j + 1) * 128, :
                            ],
                            tsb[:],
                        )

                    for d in range(DL):
                        for tg in range(8):
                            pt = xcp.tile([128, 1024], F32, tag="xct")
                            for j in range(4):
                                t = tg * 4 + j
                                # rhs = [diag(ca) | diag(ca)]: N=256 keeps f32r at
                                # 1 cycle/row; only the first 128 cols are consumed
                                nc.tensor.matmul(
                                    pt[:, j * 256 : (j + 1) * 256],
                                    x_sb[:, d, t * 128 : (t + 1) * 128].bitcast(F32R),
                                    dca2_r[:],
                                    start=True, stop=True,
                                )
                            # reduce max over c (64) for each (chunk j, b)
                            nc.vector.tensor_reduce(
                                catmax[:, :, d, tg * 4 : (tg + 1) * 4].rearrange(
                                    "p b t -> p t b"
                                ),
                                pt[:].rearrange("p (t x b c) -> p t x b c", t=4, x=2, b=2)[
                                    :, :, 0
                                ],
                                axis=mybir.AxisListType.X, op=ALU.max,
                            )
                        # cat-mean quarter q: full-d accumulation over cols
                        # q*1024..; interleaves on PE under the DVE-bound reduces
                        if d % 2 == 1:
                            q = d // 2
                            psm = cmq.tile([16, 1024], F32, tag="cmq")
                            for dd in range(DL):
                                for cj in range(2):
                                    nc.tensor.matmul(
                                        psm[:, cj * 512 : (cj + 1) * 512],
                                        cmca_r[:, dd * 16 : (dd + 1) * 16],
                                        x_sb[
                                            :, dd,
                                            q * 1024 + cj * 512 : q * 1024
                                            + (cj + 1) * 512,
                                        ].bitcast(F32R),
                                        start=(dd == 0), stop=(dd == DL - 1),
                                    )
                            nc.scalar.copy(
                                cmn_sb[:, q * 1024 : (q + 1) * 1024], psm[:]
                            )
                            nc.sync.dma_start(
                                cat_local[0:1, :, :, :].rearrange(
                                    "o b d f -> (o b d) f"
                                )[:, q * 1024 : (q + 1) * 1024],
                                cmn_sb[:, q * 1024 : (q + 1) * 1024],
                            )
                        if d == 3:
                            assemble(0)   # b0, planes 0-3
                            assemble(2)   # b1, planes 0-3
                    assemble(1)
                    assemble(3)

                if not timing:
                    nc.gpsimd.collective_compute(
                        "AllGather", ALU.bypass,
                        replica_groups=[list(range(NCORES))],
                        ins=[cat_local[:].opt()], outs=[cat_gath[:].opt()],
                    )

                # ================= phase 4: window extract + conv + sigmoid =================
                with (
                    tc.tile_pool(name="bigps", bufs=1, space="PSUM") as bpp,
                ):

                    # gathered cat: [core, stat, b, dl, hw]; half hh covers cores 4hh..4hh+3
                    # partition mapping q = (n_sub, s, b, dl) — encoded in sel matrix
                    cat_half = wsp.tile([128, HW], F32, tag="chalf")
                    wps = bpp.tile([120, HW], F32, tag="big")
                    cg4 = cat_gath[:].rearrange("n s b d f -> (n s b d) f")
                    # quarter-granular load -> round -> split-K matmul pipeline;
                    # rounding alternates DVE/ACT so quarters overlap
                    for hh in range(2):
                        for qq in range(2):
                            rows = slice(qq * 64, qq * 64 + 64)
                            nc.sync.dma_start(
                                cat_half[rows, :], cg4[hh * 128 + qq * 64 :][0:64, :]
                            )
                            if qq == 0:
                                nc.vector.tensor_copy(
                                    cat_half[rows, :].bitcast(F32R), cat_half[rows, :]
                                )
                            else:
                                nc.scalar.copy(
                                    cat_half[rows, :].bitcast(F32R), cat_half[rows, :]
                                )
                            for ch in range(NCHUNK):
                                nc.tensor.matmul(
                                    wps[:, ch * 512 : (ch + 1) * 512],
                                    sel_r[rows, hh * 120 : (hh + 1) * 120],
                                    cat_half[rows, ch * 512 : (ch + 1) * 512].bitcast(
                                        F32R
                                    ),
                                    start=(hh == 0 and qq == 0),
                                    stop=(hh == 1 and qq == 1),
                                )
                    nc.scalar.copy(
                        win_sb[0:56, PAD : PAD + H, PAD : PAD + W],
                        wps[0:56].rearrange("p (h w) -> p h w", h=H),
                    )
                    nc.vector.tensor_copy(
                        win_sb[64:120, PAD : PAD + H, PAD - 1 : PAD - 1 + W],
                        wps[64:120].rearrange("p (h w) -> p h w", h=H),
                    )


                # ===== phase 4b/5: chunk-major conv fused with gate+multiply+store =====
                with (
                    tc.tile_pool(name="convps", bufs=2, space="PSUM") as cpp,
                    tc.tile_pool(name="gateps", bufs=2, space="PSUM") as gpp,
                    tc.tile_pool(name="stage", bufs=2) as stp,
                ):
                    for chp in range(NCHUNK // 2):
                        cps = cpp.tile([16, 1024], F32, tag="cps")
                        for cj in range(2):
                            ch = chp * 2 + cj
                            h0 = ch * 8
                            for t in range(NPAIR):
                                dh, k2 = t // 4, t % 4
                                nc.tensor.matmul(
                                    cps[:, cj * 512 : (cj + 1) * 512],
                                    wc_r[:, t * 16 : (t + 1) * 16],
                                    win_sb[
                                        :, h0 + dh : h0 + dh + 8, 2 * k2 : 2 * k2 + W
                                    ],
                                    start=(t == 0), stop=(t == NPAIR - 1),
                                )
                        nc.scalar.activation(
                            sig_sb[:, chp * 1024 : (chp + 1) * 1024], cps[:],
                            ACTF.Sigmoid,
                        )
                        for d in range(DL):
                            gp = gpp.tile([128, 1024], F32, tag="gate")
                            for j in range(2):
                                nc.tensor.matmul(
                                    gp[:, j * 512 : (j + 1) * 512],
                                    bselca_r[:, d * 128 : (d + 1) * 128],
                                    sig_sb[
                                        :,
                                        (chp * 2 + j) * 512 : (chp * 2 + j + 1) * 512,
                                    ],
                                    start=True, stop=True,
                                )
                            ostage = stp.tile([128, 1024], F32, tag="stage")
                            nc.vector.tensor_tensor(
                                ostage[:],
                                x_sb[:, d, chp * 1024 : (chp + 1) * 1024],
                                gp[:],
                                op=ALU.mult,
                            )
                            nc.sync.dma_start(
                                yv[:, d, chp * 1024 : (chp + 1) * 1024], ostage[:]
                            )
                _wsp_cm.__exit__(None, None, None)


    nc.compile()
    return nc


def _host_inputs(w1, w2, w_sp):
    """Core-independent prepped weights."""
    w1blk = np.zeros((128, 16), np.float32)
    w2blk = np.zeros((16, 128), np.float32)
    for b in range(B):
        # w1blk[(b,c), (b,r)] = w1[r, c]
        w1blk[b * 64 : (b + 1) * 64, b * 8 : (b + 1) * 8] = w1.T
        # w2blk[(b,r), (b,c)] = w2[c, r]
        w2blk[b * 8 : (b + 1) * 8, b * 64 : (b + 1) * 64] = w2.T

    # paired taps: t = dh*4 + k2 covers (dh, 2*k2) in rows 0-55 and
    # (dh, 2*k2+1) in rows 56-111 (the w-shifted window copy)
    wconv = np.zeros((120, NPAIR, 16), np.float32)
    for t in range(NPAIR):
        dh, k2 = t // 4, t % 4
        for half, dw in ((0, 2 * k2), (1, 2 * k2 + 1)):
            if dw >= KS:
                continue
            for b in range(B):
                for s in range(2):
                    for de in range(DE):
                        for do in range(DL):
                            dd = de - do
                            if 0 <= dd < KS:
                                wconv[
                                    half * 64 + (b * 2 + s) * 14 + de, t, b * 8 + do
                                ] = w_sp[0, s, dd, dh, dw]
    wconv = wconv.reshape(120, NPAIR * 16)

    cmask = np.zeros((128, 8, 16), np.float32)
    for p in range(128):
        b = p // 64
        for d in range(8):
            cmask[p, d, b * 8 + d] = 1.0 / 64.0
    cmask = cmask.reshape(128, 128)

    gsum = np.zeros((128, 16), np.float32)
    for g in range(2):
        for m in range(16):
            gsum[32 * g + m, m] = 1.0

    ident = np.concatenate([np.eye(128, dtype=np.float32)] * 2, axis=1)

    bsel = np.zeros((16, 8, 128), np.float32)
    for b in range(B):
        for d in range(8):
            bsel[b * 8 + d, d, b * 64 : (b + 1) * 64] = 1.0
    bsel = bsel.reshape(16, 8 * 128)
    return w1blk, w2blk, wconv, cmask, gsum, ident, bsel


def _sel_for_core(core):
    d0 = core * DL
    sel = np.zeros((2, 128, 120), np.float32)
    for hh in range(2):
        for n_sub in range(4):
            for s in range(2):
                for b in range(B):
                    for dl in range(DL):
                        dall = (4 * hh + n_sub) * DL + dl
                        de = dall - d0 + PAD
                        q = ((n_sub * 2 + s) * 2 + b) * DL + dl
                        if 0 <= de < DE:
                            m = (b * 2 + s) * 14 + de
                            sel[hh, q, m] = 1.0
                            sel[hh, q, 64 + m] = 1.0
    return sel.transpose(1, 0, 2).reshape(128, 2 * 120).copy()


def _get_runner(reps=1):
    """Build the SPMD executable once; return a cached callable.

    Adapted from bass2jax.run_bass_via_pjrt, but keeps the jitted function
    cached across calls.
    """
    key = ("runner", reps)
    if key in _CACHED:
        return _CACHED[key]
    import jax
    import concourse.mybir as _mybir
    from jax.experimental.shard_map import shard_map
    from jax.sharding import Mesh, PartitionSpec
    from concourse.bass2jax import (
        _bass_exec_p, install_neuronx_cc_hook, partition_id_tensor,
    )

    install_neuronx_cc_hook()
    nc = _build_nc(reps=reps)

    partition_name = (
        nc.partition_id_tensor.name if nc.partition_id_tensor else None
    )
    in_names, out_names, out_avals, zero_outs = [], [], [], []
    for alloc in nc.m.functions[0].allocations:
        if not isinstance(alloc, _mybir.MemoryLocationSet):
            continue
        name = alloc.memorylocations[0].name
        if alloc.kind == "ExternalInput":
            if name != partition_name:
                in_names.append(name)
        elif alloc.kind == "ExternalOutput":
            shape = tuple(alloc.tensor_shape)
            dtype = _mybir.dt.np(alloc.dtype)
            out_names.append(name)
            out_avals.append(jax.core.ShapedArray(shape, dtype))
            zero_outs.append(np.zeros(shape, dtype))
    n_params = len(in_names)
    all_names = tuple(in_names + out_names)
    if partition_name is not None:
        all_names = all_names + (partition_name,)

    def _exec(operands):
        if partition_name is not None:
            operands = list(operands) + [partition_id_tensor()]
        return _bass_exec_p.bind(
            *operands,
            out_avals=tuple(out_avals),
            in_names=all_names,
            out_names=tuple(out_names),
            lowering_input_output_aliases=(),
            sim_require_finite=True,
            sim_require_nnan=True,
            nc=nc,
        )

    def _body(*args):
        ins = list(args[:n_params])
        outs = list(args[n_params:])
        return tuple(_exec(ins + outs))

    devices = jax.devices()[:NCORES]
    mesh = Mesh(np.asarray(devices), ("core",))
    nin = n_params + len(out_names)
    jitted = jax.jit(
        shard_map(
            _body, mesh=mesh,
            in_specs=(PartitionSpec("core"),) * nin,
            out_specs=(PartitionSpec("core"),) * len(out_names),
            check_rep=False,
        ),
        donate_argnums=tuple(range(n_params, nin)),
        keep_unused=True,
    )

    def _concat_params(in_maps):
        per_core = [[np.asarray(m[name]) for name in in_names] for m in in_maps]
        return [
            np.concatenate([per_core[c][i] for c in range(NCORES)], axis=0)
            for i in range(n_params)
        ]

    def runner(in_maps):
        concat_in = _concat_params(in_maps) + [
            np.concatenate([z] * NCORES, axis=0) for z in zero_outs
        ]
        out_arrs = jitted(*concat_in)
        out_arrs = [np.asarray(a) for a in out_arrs]
        results = []
        for c in range(NCORES):
            m = {}
            for i, name in enumerate(out_names):
                per = out_arrs[i].shape[0] // NCORES
                m[name] = out_arrs[i][c * per : (c + 1) * per]
            results.append(m)
        return results

    def time_exec(in_maps, reps=10):
        """Per-execution wall time with device-resident operands (no host I/O)."""
        import time as _time
        import jax.numpy as jnp
        from jax.sharding import NamedSharding

        shd = NamedSharding(mesh, PartitionSpec("core"))
        dev_in = [jax.device_put(a, shd) for a in _concat_params(in_maps)]
        gshapes = [
            ((NCORES * z.shape[0],) + z.shape[1:], z.dtype) for z in zero_outs
        ]
        zmaker = jax.jit(
            lambda: tuple(jnp.zeros(sh, dt) for sh, dt in gshapes),
            out_shardings=tuple(shd for _ in gshapes),
        )
        times = []
        for _ in range(reps):
            z = zmaker()
            jax.block_until_ready(z)
            jax.block_until_ready(dev_in)
            t0 = _time.perf_counter()
            out = jitted(*dev_in, *z)
            jax.block_until_ready(out)
            times.append(_time.perf_counter() - t0)
        return times

    runner.time_exec = time_exec
    _CACHED[key] = runner
    return runner


def _make_in_maps(x, w1, w2, w_sp):
    x = np.ascontiguousarray(x, np.float32)
    w1blk, w2blk, wconv, cmask, gsum, ident, bsel = _host_inputs(
        np.asarray(w1, np.float32), np.asarray(w2, np.float32),
        np.asarray(w_sp, np.float32),
    )
    in_maps = []
    for core in range(NCORES):
        in_maps.append(
            {
                "xin": np.ascontiguousarray(x[:, :, core * DL : (core + 1) * DL]),
                "w1blk": w1blk,
                "w2blk": w2blk,
                "wconv": wconv,
                "sel": _sel_for_core(core),
                "cmask": cmask,
                "gsum": gsum,
                "ident": ident,
                "bsel": bsel,
            }
        )
    return in_maps


def kernel(x, w1, w2, w_sp):
    in_maps = _make_in_maps(x, w1, w2, w_sp)
    runner = _get_runner()
    outs = runner(in_maps)
    return np.concatenate([outs[c]["y"] for c in range(NCORES)], axis=2)

